# revision 10
# baseline (speedup 1.0000x reference)
"""Causal multi-head attention (B=4, S=2048, D=2048, H=16) on 8 TRN2 NeuronCores.

Sharding: core c = 2*b + g handles batch b (of 4) and head-group g (of 2,
8 heads each).  Megatron-style: q/k/v projections are column-parallel over
the head dimension, the output projection is row-parallel; the host sums
the two partial outputs per batch and adds the bias.

All matmuls run in bf16 (same full PE rate as fp32r but half the SBUF/HBM
footprint), which lets every intermediate (qT, kT, v, oT) stay resident in
SBUF — no DRAM round-trips between the projection, attention and output
phases.  Softmax skips the max-subtraction (scores are ~N(0,1); exp cannot
overflow bf16): scores are computed transposed [sk, sq], the denominator
comes from a ones-vector matmul, and normalization is deferred to after
attn@v.  bf16 matmuls run at full rate at any width, so the causal
diagonal score matmuls narrow all the way down to 128 columns.
"""

import math

import numpy as np

B, S, D = 4, 2048, 2048
_PE = None  # set in _build_nc


def _dedupe_ldweights(nc, mybir):
    """Remove back-to-back InstLdweights with identical physical APs.

    The tile scheduler emits one InstLdweights per InstMatmult; consecutive
    matmuls that share a stationary (sub-matmuls of a >512-wide moving sweep)
    reload the same weights for ~128 PE cycles each.  A reload is redundant
    iff it has no sync_info (any rewrite of the weights tile would have
    forced a wait) and no other PE instruction touched the weight registers
    since the previous identical load.
    """
    removed = 0
    for bb in nc.m.functions[0].blocks:
        insts = bb.instructions
        keep, last_key, changed = [], None, False
        for inst in insts:
            tn = type(inst).__name__
            if tn == "InstLdweights":
                key = str(inst.ins[0])
                si = inst.sync_info
                clean = si is None or (
                    len(si.on_wait) == 0 and len(si.on_update) == 0
                )
                if key == last_key and clean:
                    removed += 1
                    changed = True
                    continue
                last_key = key
            elif tn in ("InstMatmult", "InstEventSemaphore"):
                pass  # consume weights / seq-only: weight regs unchanged
            elif inst.engine == mybir.EngineType.PE:
                last_key = None
            keep.append(inst)
        if changed:
            bb.instructions = keep
    return removed
H_TOTAL, DH = 16, 128
G = 2               # tensor-parallel head groups
HG = H_TOTAL // G   # 8 heads per group
F = HG * DH         # 1024 features per group
N_CORES = 8

_CACHE = {}


def _build_nc(iters=1):
    import concourse.mybir as mybir
    from concourse import bacc
    from concourse.tile import TileContext
    from concourse.masks import make_upper_triangular

    BF = mybir.dt.bfloat16
    F32 = mybir.dt.float32
    AF = mybir.ActivationFunctionType
    MUL = mybir.AluOpType.mult

    DT = D // 128    # 16 contraction tiles
    ST = S // 128    # 16 seq tiles
    FT = F // 128    # 8 feature tiles (= heads per group)
    SB = S // 512    # 4 seq blocks
    FB = F // 256    # 4 feature quarter-blocks (v projection)

    nc = bacc.Bacc("TRN2", target_bir_lowering=False, debug=False)
    xT = nc.dram_tensor("xT", [D, S], BF, kind="ExternalInput")
    wq = nc.dram_tensor("wq", [D, F], BF, kind="ExternalInput")
    wk = nc.dram_tensor("wk", [D, F], BF, kind="ExternalInput")
    wv = nc.dram_tensor("wv", [D, F], BF, kind="ExternalInput")
    wo = nc.dram_tensor("wo", [F, D], BF, kind="ExternalInput")
    out = nc.dram_tensor("partial", [S, D], BF, kind="ExternalOutput")

    with TileContext(nc) as tc:
        with tc.tile_pool(name="const", bufs=1) as cp:
            # Pair masks for the two diagonal j-pairs of each 512-wide sq
            # block: MP0 = [tri|ones | z128|tri|ones256] handles (j=4b,4b+1),
            # MP1 = [z256|tri|ones128 | z384|tri] handles (j=4b+2,4b+3).
            # Half h of pair p masks a diagonal at column (2p+h)*128.
            maskf = cp.tile([128, 1024], F32)
            masks = [cp.tile([128, 1024], BF, name=f"mp_{p}") for p in range(2)]
            for p in range(2):
                for hh in range(2):
                    a = 2 * p + hh
                    c0 = hh * 512
                    if a:
                        nc.gpsimd.memset(maskf[:, c0 : c0 + a * 128], 0.0)
                    make_upper_triangular(
                        nc, maskf[:, c0 + a * 128 : c0 + (a + 1) * 128],
                        val=1.0, diag=True,
                    )
                    if a < 3:
                        nc.gpsimd.memset(
                            maskf[:, c0 + (a + 1) * 128 : c0 + 512], 1.0
                        )
                nc.vector.tensor_copy(masks[p][:], maskf[:])
            o32 = cp.tile([128, 1], F32)
            nc.gpsimd.memset(o32[:], 1.0)
            ones = cp.tile([128, 1], BF)
            nc.vector.tensor_copy(ones[:], o32[:])

            for _ in range(iters):
                with tc.tile_pool(name="qkv", bufs=1) as pq:
                    # SBUF-resident intermediates for the whole iteration:
                    # qT/kT hold head h at free-index h ([dh, S] each), vv is
                    # v in natural [s, f] layout tiled by s.
                    qT = pq.tile([128, FT, S], BF)
                    kT = pq.tile([128, FT, S], BF)
                    vv = pq.tile([128, ST, F], BF)

                    # ---- phase 1: q/k/v projections ---------------------
                    # Wide psum accumulators (up to 4 banks) let one
                    # stationary serve 2-4 sub-matmuls; _dedupe_ldweights
                    # then drops the redundant weight reloads (~128 PE
                    # cycles each on hardware).
                    with tc.tile_pool(name="ph1", bufs=1) as p1:
                        xt = p1.tile([128, DT, S], BF)  # x.T fully resident

                        # v first: its x-stationary chains only need one
                        # 128-col block of x to start, and by the time the
                        # full-S q/k chains run, x is fully resident.
                        with (
                            tc.tile_pool(name="ph1v", bufs=1) as pv,
                            tc.tile_pool(name="ps1v", bufs=1, space="PSUM") as psv,
                        ):
                            wvb = pv.tile([128, DT, F], BF)
                            nc.sync.dma_start(
                                out=wvb[:],
                                in_=wv.rearrange("(t p) f -> p t f", p=128),
                            )
                            for sb in range(SB):
                                for d in range(DT):
                                    nc.sync.dma_start(
                                        out=xt[:, d, sb * 512 : (sb + 1) * 512],
                                        in_=xT[d * 128 : (d + 1) * 128, sb * 512 : (sb + 1) * 512],
                                    )
                            for st in range(ST):
                                sts = slice(st * 128, (st + 1) * 128)
                                acc = psv.tile([128, F], F32, tag="ps_v", bufs=2)
                                for d in range(DT):
                                    for i in range(F // 512):
                                        nc.tensor.matmul(
                                            acc[:, i * 512 : (i + 1) * 512],
                                            xt[:, d, sts],
                                            wvb[:, d, i * 512 : (i + 1) * 512],
                                            start=(d == 0),
                                            stop=(d == DT - 1),
                                        )
                                nc.vector.tensor_copy(vv[:, st, :], acc[:])

                        with (
                            tc.tile_pool(name="ph1w", bufs=1) as pw,
                            tc.tile_pool(name="ps1w", bufs=1, space="PSUM") as psw,
                        ):

                            def load_wqkf(f):
                                wqf = pw.tile([128, DT, 128], BF, tag="wqf", bufs=2)
                                wkf = pw.tile([128, DT, 128], BF, tag="wkf", bufs=2)
                                fs = slice(f * 128, (f + 1) * 128)
                                nc.sync.dma_start(
                                    out=wqf[:], in_=wq[:, fs].rearrange("(t p) f -> p t f", p=128)
                                )
                                nc.sync.dma_start(
                                    out=wkf[:], in_=wk[:, fs].rearrange("(t p) f -> p t f", p=128)
                                )
                                return wqf, wkf

                            wqkf0 = load_wqkf(0)
                            for f in range(FT):
                                wqf, wkf = wqkf0 if f == 0 else load_wqkf(f)
                                for w_t, dst in ((wqf, qT), (wkf, kT)):
                                    acc = psw.tile([128, S], F32, tag="ps_qk", bufs=2)
                                    for d in range(DT):
                                        for i in range(S // 512):
                                            nc.tensor.matmul(
                                                acc[:, i * 512 : (i + 1) * 512],
                                                w_t[:, d, :],
                                                xt[:, d, i * 512 : (i + 1) * 512],
                                                start=(d == 0),
                                                stop=(d == DT - 1),
                                            )
                                    nc.vector.tensor_copy(dst[:, f, :], acc[:])

                    # ---- phases 2+3 share one SBUF pool scope -----------
                    with tc.tile_pool(name="ph23", bufs=1) as p2:
                        wof = p2.tile([128, FT, D], BF)
                        oT = p2.tile([128, FT, S], BF)
                        nc.sync.dma_start(
                            out=wof[:],
                            in_=wo.rearrange("(t p) f -> p t f", p=128),
                        )

                        # ---- phase 2: causal attention per head ---------
                        with (
                            tc.tile_pool(name="ps2s", bufs=1, space="PSUM") as ps2s,
                            tc.tile_pool(name="ps2o", bufs=1, space="PSUM") as ps2o,
                        ):
                            # Software-pipelined by two j-pairs: pair p's av/l
                            # matmuls are emitted after pair p+2's score
                            # matmuls, so the PE never sits behind p's exp
                            # (ACT) or the diagonal mask multiply (DVE).
                            DEPTH = 2
                            pend = []  # (pt, h, acc_o, acc_l, j0, jmax)
                            epilogue = None  # accumulators of a finished block

                            def flush_pending():
                                nonlocal epilogue
                                if not pend:
                                    return
                                pt_, h_, acc_o_, acc_l_, j0_, jmax_ = pend.pop(0)
                                hs_ = slice(h_ * 128, (h_ + 1) * 128)
                                # av pair first, then the l pair, so the two
                                # identical `ones` loads sit adjacent and the
                                # second dedupes away.
                                for dst_, stat_ in ((acc_o_, "v"), (acc_l_, "1")):
                                    for hh in range(2):
                                        j = j0_ + hh
                                        # columns below the causal diagonal
                                        # are zero in pt — skip them (bf16
                                        # runs full rate at any width),
                                        # except the group-closing j==jmax
                                        # matmul which must span the full
                                        # accumulation region.
                                        a = j - (jmax_ - 3)
                                        c0 = a * 128 if a in (1, 2) else 0
                                        pslice = pt_[:, hh * 512 + c0 : (hh + 1) * 512]
                                        nc.tensor.matmul(
                                            dst_[:, c0:512],
                                            vv[:, j, hs_] if stat_ == "v" else ones[:],
                                            pslice,
                                            start=(j == 0), stop=(j == jmax_),
                                        )
                                if j0_ + 1 == jmax_:  # block finished
                                    epilogue = (acc_o_, acc_l_)

                            def flush_epilogue(h_, bs_):
                                nonlocal epilogue
                                assert epilogue is not None
                                acc_o_, acc_l_ = epilogue
                                epilogue = None
                                linv = p2.tile([1, 512], F32, tag="linv", bufs=2)
                                nc.vector.reciprocal(linv[:], acc_l_[:])
                                linb = p2.tile([128, 512], F32, tag="linb", bufs=2)
                                nc.gpsimd.partition_broadcast(linb[:], linv[:])
                                nc.vector.tensor_tensor(
                                    out=oT[:, h_, bs_], in0=acc_o_[:], in1=linb[:], op=MUL
                                )

                            blocks = []  # (h, bs) epilogue coords in flight
                            for h in range(HG):
                                for b in range(SB):
                                    bs = slice(b * 512, (b + 1) * 512)
                                    acc_o = ps2o.tile([128, 512], F32, tag="ps_o", bufs=2)
                                    acc_l = ps2o.tile([1, 512], F32, tag="ps_l", bufs=2)
                                    jmax = 4 * b + 3
                                    for jp in range(2 * b + 2):
                                        j0 = 2 * jp
                                        sc = ps2s.tile([128, 1024], F32, tag="ps_s", bufs=2)
                                        for hh in range(2):
                                            j = j0 + hh
                                            # causal: columns sq < j*128 are
                                            # dead; the skipped psum region
                                            # holds stale (bounded) scores;
                                            # exp of it is finite and the
                                            # pair mask zeroes it.
                                            a = j - 4 * b
                                            c0 = a * 128 if a in (1, 2, 3) else 0
                                            nc.tensor.matmul(
                                                sc[:, hh * 512 + c0 : (hh + 1) * 512],
                                                kT[:, h, j * 128 : (j + 1) * 128],
                                                qT[:, h, b * 512 + c0 : (b + 1) * 512],
                                                start=True,
                                                stop=True,
                                            )
                                        pt = p2.tile([128, 1024], BF, tag="pt", bufs=4)
                                        nc.scalar.activation(pt[:], sc[:], AF.Exp)
                                        if j0 >= 4 * b:  # diagonal pair
                                            nc.vector.tensor_tensor(
                                                out=pt[:],
                                                in0=pt[:],
                                                in1=masks[jp - 2 * b][:],
                                                op=MUL,
                                            )
                                        pend.append((pt, h, acc_o, acc_l, j0, jmax))
                                        if len(pend) > DEPTH:
                                            flush_pending()
                                            if epilogue is not None:
                                                flush_epilogue(*blocks.pop(0))
                                    blocks.append((h, bs))
                            while pend:
                                flush_pending()
                                if epilogue is not None:
                                    flush_epilogue(*blocks.pop(0))

                        # ---- phase 3: output projection -----------------
                        with tc.tile_pool(name="ps3", bufs=1, space="PSUM") as ps3:
                            for st in range(ST):
                                sts = slice(st * 128, (st + 1) * 128)
                                acc = ps3.tile([128, S], F32, tag="ps_p", bufs=2)
                                for f in range(FT):
                                    for i in range(D // 512):
                                        nc.tensor.matmul(
                                            acc[:, i * 512 : (i + 1) * 512],
                                            oT[:, f, sts],
                                            wof[:, f, i * 512 : (i + 1) * 512],
                                            start=(f == 0),
                                            stop=(f == FT - 1),
                                        )
                                po = p2.tile([128, S], BF, tag="po", bufs=2)
                                nc.vector.tensor_copy(po[:], acc[:])
                                nc.sync.dma_start(out=out[sts, :], in_=po[:])

    _dedupe_ldweights(nc, mybir)
    nc.compile()
    return nc


def _get_nc(iters=1):
    key = ("nc", iters)
    if key not in _CACHE:
        _CACHE[key] = _build_nc(iters)
    return _CACHE[key]


def make_in_maps(x, Wq, Wk, Wv, Wo):
    import ml_dtypes

    BF = ml_dtypes.bfloat16
    scale = np.float32(1.0 / math.sqrt(DH))
    xTs = [np.ascontiguousarray(x[b].T).astype(BF) for b in range(B)]
    in_maps = []
    for c in range(N_CORES):
        b, g = divmod(c, G)
        gs = slice(g * F, (g + 1) * F)
        in_maps.append(
            {
                "xT": xTs[b],
                "wq": (np.ascontiguousarray(Wq[gs, :].T) * scale).astype(BF),
                "wk": np.ascontiguousarray(Wk[gs, :].T).astype(BF),
                "wv": np.ascontiguousarray(Wv[gs, :].T).astype(BF),
                "wo": np.ascontiguousarray(Wo[:, gs].T).astype(BF),
            }
        )
    return in_maps


def kernel(x, Wq, Wk, Wv, Wo, bo):
    from concourse.bass_utils import run_bass_kernel_spmd

    x = np.asarray(x, dtype=np.float32)
    Wq = np.asarray(Wq, dtype=np.float32)
    Wk = np.asarray(Wk, dtype=np.float32)
    Wv = np.asarray(Wv, dtype=np.float32)
    Wo = np.asarray(Wo, dtype=np.float32)
    bo = np.asarray(bo, dtype=np.float32)

    nc = _get_nc()
    in_maps = make_in_maps(x, Wq, Wk, Wv, Wo)
    res = run_bass_kernel_spmd(nc, in_maps, list(range(N_CORES)))
    out = np.empty((B, S, D), dtype=np.float32)
    for b in range(B):
        out[b] = (
            res.results[2 * b]["partial"].astype(np.float32)
            + res.results[2 * b + 1]["partial"].astype(np.float32)
            + bo
        )
    return out
